# revision 1
# baseline (speedup 1.0000x reference)
"""CBOW forward kernel for one TRN2 chip (8 NeuronCores), tensor-parallel on vocab.

Math (matches the reference):
    embed[b, c, :] = emb_W.T[contexts[b, c]] + emb_b          # gather
    out = embed.reshape(B, CTX*EMB) @ fc_W.T + fc_b           # [B, VOCAB]

Distribution: vocab dim sharded 8 ways (fc_W rows / fc_b / output columns);
contexts + emb table replicated so the gather is fully local — no collectives.
Each core computes out_shard.T = fc_W_shard @ embed.T  ->  [VSHARD, B], and the
host concatenates the shards and returns the transposed view.

Per-core device schedule:
  1. one DMA for indices, 4 indirect-DMA gathers of 256 B emb rows -> raw
     [128 batch, 512 feat] per batch tile
  2. PE transposes raw 128x128 blocks -> embT (K-major, bf16) -- the moving
     matmul operand
  3. main loop over 98 vocab tiles: 4 ldweights (fc bf16) + 16 matmuls
     (K=512 accumulated in PSUM) per tile, bias-add fused into the
     PSUM->SBUF drain (scalar/vector engines), 1 MiB contiguous output DMA

emb_b and fc_b are folded on the host into one effective bias
fc_be = fc_W @ tile(emb_b, CTX) + fc_b  (pure weight preprocessing).
"""

import os

import numpy as np
import ml_dtypes

import concourse.bacc as bacc
import concourse.bass as bass
import concourse.mybir as mybir
import concourse.tile as tile
from concourse.bass_utils import run_bass_kernel_spmd
from concourse.masks import make_identity

# Problem shape (hardcoded per harness contract).
VOCAB = 100000
CTX = 8
EMB = 64
BATCH = 2048
K = CTX * EMB            # 512 contraction dim
NCORES = 8
VSHARD = 12544           # 98 * 128, vocab cols per core (padded)
VPAD = VSHARD * NCORES   # 100352
NVT = VSHARD // 128      # 98 vocab tiles per core
VCHUNK = 7               # vocab tiles per fc_W DMA chunk
NCHUNK = NVT // VCHUNK   # 14
CHUNK_COLS = VCHUNK * 128  # 896
NBT = BATCH // 128       # 16 batch tiles
NBC = BATCH // 512       # 4 batch chunks (psum banks per vocab tile)

F32 = mybir.dt.float32
BF16 = mybir.dt.bfloat16
I32 = mybir.dt.int32
OUT_DT = BF16            # output quantization: rel err ~2e-3 << 2e-2 gate

_CACHE = {}


def _install_trace_hook():
    """Provide the missing antenv.axon_hooks module so trace=True works.

    The agent image's antenv lacks axon_hooks; recreate it and install the
    ctypes NTFF hook from trn_boot. Degrades silently on any failure.
    """
    import sys
    import types

    try:
        if "antenv.axon_hooks" not in sys.modules:
            mod = types.ModuleType("antenv.axon_hooks")
            mod._hook = None
            mod.set_axon_ntff_profile_hook = lambda h: setattr(mod, "_hook", h)
            mod.get_axon_ntff_profile_hook = lambda: mod._hook
            sys.modules["antenv.axon_hooks"] = mod
            import antenv

            antenv.axon_hooks = mod
        mod = sys.modules["antenv.axon_hooks"]
        if mod.get_axon_ntff_profile_hook() is None:
            if "/root/.axon_site/trn_agent_boot" not in sys.path:
                sys.path.insert(0, "/root/.axon_site/trn_agent_boot")
            import trn_boot

            mod.set_axon_ntff_profile_hook(
                trn_boot._ntff_profile_via_ctypes("/opt/axon/libaxon_pjrt.so")
            )
        return True
    except Exception as e:  # pragma: no cover
        print(f"trace hook install failed: {type(e).__name__}: {e}")
        return False


NJL = BATCH * CTX // NCORES // 128   # 16 gather calls per core


def _build_nc(probe_1core=False):
    nc = bacc.Bacc(
        "TRN2", target_bir_lowering=False, debug=False, num_devices=NCORES
    )
    idx_my = nc.declare_dram_parameter("idx_my", [128, NJL], I32, isOutput=False)
    emb_wt = nc.declare_dram_parameter("emb_wt", [VOCAB, EMB], F32, isOutput=False)
    fc_w = nc.declare_dram_parameter(
        "fc_w", [NCHUNK, 128, 4, CHUNK_COLS], BF16, isOutput=False
    )
    fc_be = nc.declare_dram_parameter("fc_be", [128, NVT], F32, isOutput=False)
    out = nc.declare_dram_parameter("out", [VSHARD, BATCH], OUT_DT, isOutput=True)

    with tile.TileContext(nc) as tc:
        with tc.tile_pool(name="const", bufs=1) as const:
            idx_sb = const.tile([128, NJL], I32, tag="idx", name="idx_sb")
            nc.sync.dma_start(out=idx_sb[:], in_=idx_my[:])
            ident = const.tile([128, 128], F32, tag="ident", name="ident")
            make_identity(nc, ident[:])
            fcbe_sb = const.tile([128, NVT], F32, tag="fcbe", name="fcbe_sb")
            nc.sync.dma_start(out=fcbe_sb[:], in_=fc_be[:])
            # warm the ACT Identity table before the main loop needs it
            actwarm = const.tile([128, 1], F32, tag="actwarm", name="actwarm")
            nc.scalar.activation(
                out=actwarm[:],
                in_=fcbe_sb[:, 0:1],
                func=mybir.ActivationFunctionType.Identity,
                bias=fcbe_sb[:, 0:1],
            )

            # Local gather of this core's 1/8 of the batch (one emb row per
            # partition per call): raw_loc[p, (ml*8+c)*64+e] for local batch
            # tiles ml in {0,1} (global m = 2s+ml).
            raw_loc = const.tile([128, NJL * EMB], F32, tag="rawloc", name="raw_loc")
            for jl in range(NJL):
                nc.gpsimd.indirect_dma_start(
                    out=raw_loc[:, jl * EMB : (jl + 1) * EMB],
                    out_offset=None,
                    in_=emb_wt[:],
                    in_offset=bass.IndirectOffsetOnAxis(
                        ap=idx_sb[:, jl : jl + 1], axis=0
                    ),
                )

            # Transpose the local slice to K-major bf16 BEFORE the exchange,
            # then AllGather the transposed slices: embT[k][q, b] =
            # embed_flat[b, k*128+q] with b = s*256 + (local col).
            LB = 128 * 2  # local batch count
            embT = []
            for k in range(4):
                t = const.tile([128, BATCH], BF16, tag=f"embT{k}", name=f"embT{k}")
                embT.append(t)
            embT_loc = const.tile([128, 4 * LB], BF16, tag="embTloc", name="embT_loc")
            with tc.tile_pool(name="tpsum", bufs=4, space="PSUM") as tps:
                for ml in range(2):
                    for k in range(4):
                        ps = tps.tile([128, 128], F32, tag="tps", name="tps")
                        nc.tensor.transpose(
                            ps[:],
                            raw_loc[:, ml * K + k * 128 : ml * K + (k + 1) * 128],
                            ident[:],
                        )
                        nc.vector.tensor_copy(
                            out=embT_loc[
                                :, k * LB + ml * 128 : k * LB + (ml + 1) * 128
                            ],
                            in_=ps[:],
                        )
            with tc.tile_pool(name="dramp", bufs=1, space="DRAM") as dramp:
                ag_in = dramp.tile([128, 4 * LB], BF16, tag="agin", name="ag_in")
                ag_out = dramp.tile(
                    [NCORES, 128, 4 * LB], BF16, tag="agout", name="ag_out",
                    addr_space="Shared",
                )
                nc.gpsimd.dma_start(out=ag_in[:], in_=embT_loc[:])
                if probe_1core:
                    # timing probe only: skip the collective, fill embT with
                    # the local slice (values wrong, timing representative)
                    for k in range(4):
                        for rep in range(NCORES):
                            nc.sync.dma_start(
                                out=embT[k][:, rep * LB : (rep + 1) * LB],
                                in_=ag_in[:, k * LB : (k + 1) * LB],
                            )
                else:
                    nc.gpsimd.collective_compute(
                        "AllGather",
                        mybir.AluOpType.bypass,
                        replica_groups=[list(range(NCORES))],
                        ins=[ag_in[:]],
                        outs=[ag_out[:]],
                    )
                    for k in range(4):
                        nc.sync.dma_start(
                            out=embT[k][:],
                            in_=ag_out[:, :, k * LB : (k + 1) * LB].rearrange(
                                "s p c -> p s c"
                            ),
                        )

            # Main loop: out.T[v*128:(v+1)*128, :] for 98 vocab tiles.
            with (
                tc.tile_pool(name="fcp", bufs=2) as fcp,
                tc.tile_pool(name="outp", bufs=4) as outp,
                tc.tile_pool(name="mpsum", bufs=2, space="PSUM") as mps,
            ):
                for ci in range(NCHUNK):
                    fct = fcp.tile([128, 4, CHUNK_COLS], BF16, tag="fct", name="fct")
                    nc.sync.dma_start(out=fct[:], in_=fc_w[ci])
                    for vt in range(VCHUNK):
                        v = ci * VCHUNK + vt
                        pss = [
                            mps.tile([128, 512], F32, tag=f"ps{bc}", name=f"ps{bc}")
                            for bc in range(NBC)
                        ]
                        for k in range(4):
                            lhsT = fct[:, k, vt * 128 : (vt + 1) * 128]
                            for bc in range(NBC):
                                nc.tensor.matmul(
                                    out=pss[bc][:],
                                    lhsT=lhsT,
                                    rhs=embT[k][:, bc * 512 : (bc + 1) * 512],
                                    start=(k == 0),
                                    stop=(k == 3),
                                )
                        osb = outp.tile([128, BATCH], OUT_DT, tag="osb", name="osb")
                        for bc in range(NBC):
                            if bc < 2:
                                nc.scalar.activation(
                                    out=osb[:, bc * 512 : (bc + 1) * 512],
                                    in_=pss[bc][:],
                                    func=mybir.ActivationFunctionType.Identity,
                                    bias=fcbe_sb[:, v : v + 1],
                                )
                            else:
                                nc.vector.tensor_scalar_add(
                                    out=osb[:, bc * 512 : (bc + 1) * 512],
                                    in0=pss[bc][:],
                                    scalar1=fcbe_sb[:, v : v + 1],
                                )
                        nc.sync.dma_start(
                            out=out[v * 128 : (v + 1) * 128, :], in_=osb[:]
                        )
    nc.compile()
    return nc


def _prep_inputs(contexts, emb_W, emb_b, fc_W, fc_b):
    contexts = np.asarray(contexts)
    emb_W = np.asarray(emb_W, dtype=np.float32)
    emb_b = np.asarray(emb_b, dtype=np.float32)
    fc_W = np.asarray(fc_W, dtype=np.float32)
    fc_b = np.asarray(fc_b, dtype=np.float32)

    # idx2d[j, p] = contexts[(j//8)*128 + p, j%8] with j = m*8+c; core s
    # gathers columns j in [16s, 16(s+1)) for its 1/8 of the batch.
    idx2d = (
        contexts.astype(np.int64).reshape(NBT, 128, CTX).transpose(0, 2, 1)
        .reshape(NBT * CTX, 128)
    )
    emb_wt = np.ascontiguousarray(emb_W.T)  # [VOCAB, 64] f32

    # effective bias: fc_be = fc_W @ tile(emb_b, CTX) + fc_b  (padded)
    emb_b_t = np.tile(emb_b, CTX)
    fc_be_full = (
        fc_W.astype(np.float64) @ emb_b_t.astype(np.float64)
        + fc_b.astype(np.float64)
    ).astype(np.float32)
    fc_be_pad = np.zeros(VPAD, dtype=np.float32)
    fc_be_pad[:VOCAB] = fc_be_full

    # fc_W.T padded to VPAD cols, bf16, chunked per-core layout
    fcT = np.zeros((K, VPAD), dtype=np.float32)
    fcT[:, :VOCAB] = fc_W.T
    fcT = fcT.astype(ml_dtypes.bfloat16)

    in_maps = []
    for s in range(NCORES):
        shard = fcT[:, s * VSHARD : (s + 1) * VSHARD]
        fc_host = np.ascontiguousarray(
            shard.reshape(4, 128, NCHUNK, CHUNK_COLS).transpose(2, 1, 0, 3)
        )
        be = np.ascontiguousarray(
            fc_be_pad[s * VSHARD : (s + 1) * VSHARD].reshape(NVT, 128).T
        )
        idx_my = np.ascontiguousarray(
            idx2d[s * NJL : (s + 1) * NJL, :].T.astype(np.int32)
        )
        in_maps.append(
            {"idx_my": idx_my, "emb_wt": emb_wt, "fc_w": fc_host, "fc_be": be}
        )
    return in_maps


def kernel(contexts, emb_W, emb_b, fc_W, fc_b):
    if "nc" not in _CACHE:
        _CACHE["nc"] = _build_nc()
    nc = _CACHE["nc"]
    in_maps = _prep_inputs(contexts, emb_W, emb_b, fc_W, fc_b)
    trace = bool(int(os.environ.get("KERNEL_TRACE", "0")))
    if trace:
        trace = _install_trace_hook()
    res = run_bass_kernel_spmd(
        nc, in_maps, core_ids=list(range(NCORES)), trace=trace
    )
    _CACHE["last_exec_time_ns"] = res.exec_time_ns
    full = np.concatenate(
        [np.asarray(r["out"]).astype(np.float32) for r in res.results], axis=0
    )
    return full[:VOCAB].T



# revision 4
# speedup vs baseline: 1.2907x; 1.2907x over previous
"""CBOW forward kernel for one TRN2 chip (8 NeuronCores), tensor-parallel on vocab.

Math (matches the reference):
    embed[b, c, :] = emb_W.T[contexts[b, c]] + emb_b          # gather
    out = embed.reshape(B, CTX*EMB) @ fc_W.T + fc_b           # [B, VOCAB]

Distribution: vocab dim sharded 8 ways (fc_W rows / fc_b / output columns).
contexts + emb table replicated; EVERY core gathers the full batch locally so
there are NO collectives (the old AllGather cost ~68us of PE idle).

Numerics: the big matmul runs in fp8 e4m3 with perf_mode=DoubleRow (2 fp8
weights per PE cell -> 2x bf16 throughput). Both operands are scaled by 2^9,
clipped to +-240 (TRN e4m3 max) and RNE-quantized; PSUM accumulates fp32; the
drain multiplies by 2^-18 and adds the exact f32 effective bias
fc_be = fc_W @ tile(emb_b, CTX) + fc_b. Measured end-to-end rel err vs the
f32 reference: 1.39e-2 (gate 2e-2). emb_b folding into fc_be is exact.

Per-core schedule:
  1. 4 batched indirect-DMA gathers (512 rows/call, bf16 table) -> raw16
     [128 batch, 8192] = full 2048x8 gather, one call per 512-batch chunk
  2. fc_W fp8 shard (6.3 MB) streamed into SBUF once (resident), 14 chunks
  3. per batch chunk bc: 16 PE transposes (bf16) + DVE copy-casts build the
     pair-interleaved fp8 moving operand embT8[kg][k%256 part, batch, pair];
     then 98 vocab tiles x 2 DoubleRow matmuls (K=512 = 2 groups of 256)
     into one PSUM bank each; scalar/vector alternate the scale+bias drain;
     contiguous 128KB output DMA per (bc, vocab tile).
"""

import os

import numpy as np
import ml_dtypes

import concourse.bacc as bacc
import concourse.bass as bass
import concourse.mybir as mybir
import concourse.tile as tile
from concourse.bass_utils import run_bass_kernel_spmd
from concourse.masks import make_identity

# Problem shape (hardcoded per harness contract).
VOCAB = 100000
CTX = 8
EMB = 64
BATCH = 2048
K = CTX * EMB            # 512 contraction dim
NCORES = 8
VSHARD = 12544           # 98 * 128, vocab cols per core (padded)
VPAD = VSHARD * NCORES   # 100352
NVT = VSHARD // 128      # 98 vocab tiles per core
VCHUNK = 7               # vocab tiles per fc DMA chunk
NCHUNK = NVT // VCHUNK   # 14
CHUNK_COLS = VCHUNK * 128  # 896
NBT = BATCH // 128       # 16 batch tiles
NBC = 4                  # batch chunks (512 each) — outer loop
MPB = NBT // NBC         # batch tiles per chunk

F32 = mybir.dt.float32
BF16 = mybir.dt.bfloat16
FP8 = mybir.dt.float8e4
I32 = mybir.dt.int32
OUT_DT = BF16

E4NP = ml_dtypes.float8_e4m3   # TRN FP8_EXP4 semantics (max 240, inf above)
BFNP = ml_dtypes.bfloat16

SE = 2.0 ** 9            # embedding scale (pow2: exact descale)
SW = 2.0 ** 9            # fc weight scale
DESCALE = 1.0 / (SE * SW)

_CACHE = {}


def _install_trace_hook():
    """Provide the missing antenv.axon_hooks module so trace=True works."""
    import sys
    import types

    try:
        if "antenv.axon_hooks" not in sys.modules:
            mod = types.ModuleType("antenv.axon_hooks")
            mod._hook = None
            mod.set_axon_ntff_profile_hook = lambda h: setattr(mod, "_hook", h)
            mod.get_axon_ntff_profile_hook = lambda: mod._hook
            sys.modules["antenv.axon_hooks"] = mod
            import antenv

            antenv.axon_hooks = mod
        mod = sys.modules["antenv.axon_hooks"]
        if mod.get_axon_ntff_profile_hook() is None:
            if "/root/.axon_site/trn_agent_boot" not in sys.path:
                sys.path.insert(0, "/root/.axon_site/trn_agent_boot")
            import trn_boot

            mod.set_axon_ntff_profile_hook(
                trn_boot._ntff_profile_via_ctypes("/opt/axon/libaxon_pjrt.so")
            )
        return True
    except Exception as e:  # pragma: no cover
        print(f"trace hook install failed: {type(e).__name__}: {e}")
        return False


def _build_nc():
    nc = bacc.Bacc(
        "TRN2", target_bir_lowering=False, debug=False, num_devices=NCORES
    )
    # idx_all[p, j] = contexts[(j//8)*128 + p, j%8]  (j = m*8+c), same all cores
    idx_all = nc.declare_dram_parameter("idx_all", [128, 128], I32, isOutput=False)
    emb_wt = nc.declare_dram_parameter("emb_wt", [VOCAB, EMB], BF16, isOutput=False)
    # fc_w[ci, i, kg, par, w]: e4m3( SW * fc_W.T[kg*256+par*128+i, shard v] )
    fc_w = nc.declare_dram_parameter(
        "fc_w", [NCHUNK, 128, 2, 2, CHUNK_COLS], FP8, isOutput=False
    )
    fc_be = nc.declare_dram_parameter("fc_be", [128, NVT], F32, isOutput=False)
    out = nc.declare_dram_parameter(
        "out", [NBC, VSHARD, 512], OUT_DT, isOutput=True
    )

    with tile.TileContext(nc) as tc:
        with (
            tc.tile_pool(name="const", bufs=1) as const,
            tc.tile_pool(name="tpsum", bufs=2, space="PSUM") as tps,
            tc.tile_pool(name="mpsum", bufs=6, space="PSUM") as mps,
            tc.tile_pool(name="outp", bufs=8) as outp,
        ):
            idx_sb = const.tile([128, 128], I32, tag="idx", name="idx_sb")
            nc.sync.dma_start(out=idx_sb[:], in_=idx_all[:])
            fcbe_sb = const.tile([128, NVT], F32, tag="fcbe", name="fcbe_sb")
            nc.sync.dma_start(out=fcbe_sb[:], in_=fc_be[:])
            ident = const.tile([128, 128], BF16, tag="ident", name="ident")
            make_identity(nc, ident[:])
            # warm the ACT Identity table before the main loop needs it
            actwarm = const.tile([128, 1], F32, tag="actwarm", name="actwarm")
            nc.scalar.activation(
                out=actwarm[:],
                in_=fcbe_sb[:, 0:1],
                func=mybir.ActivationFunctionType.Identity,
                bias=fcbe_sb[:, 0:1],
            )

            # resident fc weights: [128 i, ci, kg, par, w] fp8 (49 KB/part)
            fcsb = const.tile(
                [128, NCHUNK, 2, 2, CHUNK_COLS], FP8, tag="fcsb", name="fcsb"
            )
            for ci in range(NCHUNK):
                nc.sync.dma_start(out=fcsb[:, ci], in_=fc_w[ci])

            # full-batch gather: raw16[p, j*64+e] = emb_wt[idx[p, j], e].
            # HW honors only ONE offset column per indirect call (multi-col
            # offset APs gather consecutive rows — verified on silicon), so
            # issue 128 single-column calls; bc0 needs only the first 32.
            raw16 = const.tile([128, NBT * K], BF16, tag="raw16", name="raw16")
            for j in range(NBT * CTX):
                nc.gpsimd.indirect_dma_start(
                    out=raw16[:, j * EMB : (j + 1) * EMB],
                    out_offset=None,
                    in_=emb_wt[:],
                    in_offset=bass.IndirectOffsetOnAxis(
                        ap=idx_sb[:, j : j + 1], axis=0
                    ),
                )

            # pair-interleaved fp8 moving operand:
            # embT8[kg][i, n, par] = e4m3(embed_scaled[kg*256+par*128+i, n])
            embT8 = [
                const.tile([128, BATCH, 2], FP8, tag=f"embT8{g}", name=f"embT8{g}")
                for g in range(2)
            ]

            for bc in range(NBC):
                # build embT8[:, bc*512:(bc+1)*512, :] from raw16
                for ml in range(MPB):
                    m = bc * MPB + ml
                    for kb in range(4):
                        ps = tps.tile([128, 1024], BF16, tag="tps", name="tps")
                        nc.tensor.transpose(
                            ps[:, 0:128],
                            raw16[:, m * K + kb * 128 : m * K + (kb + 1) * 128],
                            ident[:],
                        )
                        kg, par = kb // 2, kb % 2
                        nc.vector.tensor_copy(
                            out=embT8[kg][:, m * 128 : (m + 1) * 128, par],
                            in_=ps[:, 0:128],
                        )

                # 98 vocab tiles for this batch chunk
                for v in range(NVT):
                    ci, vt = v // VCHUNK, v % VCHUNK
                    psm = mps.tile([128, 512], F32, tag="mps", name="mps")
                    for kg in range(2):
                        nc.tensor.matmul(
                            out=psm[:],
                            lhsT=fcsb[:, ci, kg, :, vt * 128 : (vt + 1) * 128],
                            rhs=embT8[kg][
                                :, bc * 512 : (bc + 1) * 512, :
                            ].rearrange("p n t -> p t n"),
                            start=(kg == 0),
                            stop=(kg == 1),
                            perf_mode=mybir.MatmulPerfMode.DoubleRow,
                        )
                    osb = outp.tile([128, 512], OUT_DT, tag="osb", name="osb")
                    if v % 2 == 0:
                        nc.scalar.activation(
                            out=osb[:],
                            in_=psm[:],
                            func=mybir.ActivationFunctionType.Identity,
                            bias=fcbe_sb[:, v : v + 1],
                            scale=DESCALE,
                        )
                    else:
                        nc.vector.tensor_scalar(
                            out=osb[:],
                            in0=psm[:],
                            scalar1=DESCALE,
                            scalar2=fcbe_sb[:, v : v + 1],
                            op0=mybir.AluOpType.mult,
                            op1=mybir.AluOpType.add,
                        )
                    nc.sync.dma_start(
                        out=out[bc, v * 128 : (v + 1) * 128, :], in_=osb[:]
                    )
    nc.compile()
    return nc


def _prep_inputs(contexts, emb_W, emb_b, fc_W, fc_b):
    contexts = np.asarray(contexts)
    emb_W = np.asarray(emb_W, dtype=np.float32)
    emb_b = np.asarray(emb_b, dtype=np.float32)
    fc_W = np.asarray(fc_W, dtype=np.float32)
    fc_b = np.asarray(fc_b, dtype=np.float32)

    # idx_all[p, j] = contexts[(j//8)*128 + p, j%8]
    idx2d = (
        contexts.astype(np.int64).reshape(NBT, 128, CTX).transpose(0, 2, 1)
        .reshape(NBT * CTX, 128)
    )
    idx_all = np.ascontiguousarray(idx2d.T.astype(np.int32))

    # scaled bf16 embedding table (device casts bf16 -> e4m3 during drain copy)
    emb_wt = np.ascontiguousarray(
        np.clip(emb_W.T * SE, -240.0, 240.0).astype(BFNP)
    )

    # effective bias: fc_be = fc_W @ tile(emb_b, CTX) + fc_b  (exact, padded)
    emb_b_t = np.tile(emb_b, CTX)
    fc_be_full = (
        fc_W.astype(np.float64) @ emb_b_t.astype(np.float64)
        + fc_b.astype(np.float64)
    ).astype(np.float32)
    fc_be_pad = np.zeros(VPAD, dtype=np.float32)
    fc_be_pad[:VOCAB] = fc_be_full

    # fc_W.T scaled/quantized to e4m3, padded to VPAD cols
    fcT = np.zeros((K, VPAD), dtype=np.float32)
    fcT[:, :VOCAB] = fc_W.T
    fcq = np.clip(fcT * SW, -240.0, 240.0).astype(E4NP)

    in_maps = []
    for s in range(NCORES):
        shard = fcq[:, s * VSHARD : (s + 1) * VSHARD]
        # [k=kg*256+par*128+i, v=ci*896+w] -> [ci, i, kg, par, w]
        fc_host = np.ascontiguousarray(
            shard.reshape(2, 2, 128, NCHUNK, CHUNK_COLS).transpose(3, 2, 0, 1, 4)
        )
        be = np.ascontiguousarray(
            fc_be_pad[s * VSHARD : (s + 1) * VSHARD].reshape(NVT, 128).T
        )
        in_maps.append(
            {"idx_all": idx_all, "emb_wt": emb_wt, "fc_w": fc_host, "fc_be": be}
        )
    return in_maps


def kernel(contexts, emb_W, emb_b, fc_W, fc_b):
    if "nc" not in _CACHE:
        _CACHE["nc"] = _build_nc()
    nc = _CACHE["nc"]
    in_maps = _prep_inputs(contexts, emb_W, emb_b, fc_W, fc_b)
    trace = bool(int(os.environ.get("KERNEL_TRACE", "0")))
    if trace:
        trace = _install_trace_hook()
    res = run_bass_kernel_spmd(
        nc, in_maps, core_ids=list(range(NCORES)), trace=trace
    )
    _CACHE["last_exec_time_ns"] = res.exec_time_ns
    # out[s][bc, v*128+r, j] = logits[bc*512+j, s*VSHARD + v*128+r]
    full = np.empty((BATCH, VPAD), dtype=np.float32)
    for s, r in enumerate(res.results):
        o = np.asarray(r["out"]).astype(np.float32)  # [NBC, VSHARD, 512]
        for bc in range(NBC):
            full[bc * 512 : (bc + 1) * 512, s * VSHARD : (s + 1) * VSHARD] = o[
                bc
            ].T
    return np.ascontiguousarray(full[:, :VOCAB])


# revision 6
# speedup vs baseline: 1.8203x; 1.4104x over previous
"""CBOW forward kernel for one TRN2 chip (8 NeuronCores), tensor-parallel on vocab.

Math (matches the reference):
    embed[b, c, :] = emb_W.T[contexts[b, c]] + emb_b          # gather
    out = embed.reshape(B, CTX*EMB) @ fc_W.T + fc_b           # [B, VOCAB]

Distribution: vocab dim sharded 8 ways (fc_W rows / fc_b / output columns).
contexts + emb table replicated; EVERY core gathers the full batch locally so
there are NO collectives (the old AllGather cost ~68us of PE idle).

Numerics: the big matmul runs in fp8 e4m3 with perf_mode=DoubleRow (2 fp8
weights per PE cell -> 2x bf16 throughput). Both operands are scaled by 2^9,
clipped to +-240 (TRN e4m3 max) and RNE-quantized; PSUM accumulates fp32; the
drain multiplies by 2^-18 and adds the exact f32 effective bias
fc_be = fc_W @ tile(emb_b, CTX) + fc_b. Measured end-to-end rel err vs the
f32 reference: 1.39e-2 (gate 2e-2). emb_b folding into fc_be is exact.

Per-core schedule:
  1. 4 batched indirect-DMA gathers (512 rows/call, bf16 table) -> raw16
     [128 batch, 8192] = full 2048x8 gather, one call per 512-batch chunk
  2. fc_W fp8 shard (6.3 MB) streamed into SBUF once (resident), 14 chunks
  3. per batch chunk bc: 16 PE transposes (bf16) + DVE copy-casts build the
     pair-interleaved fp8 moving operand embT8[kg][k%256 part, batch, pair];
     then 98 vocab tiles x 2 DoubleRow matmuls (K=512 = 2 groups of 256)
     into one PSUM bank each; scalar/vector alternate the scale+bias drain;
     contiguous 128KB output DMA per (bc, vocab tile).
"""

import os

import numpy as np
import ml_dtypes

import concourse.bacc as bacc
import concourse.bass as bass
import concourse.mybir as mybir
import concourse.tile as tile
from concourse.bass_utils import run_bass_kernel_spmd
from concourse.masks import make_identity

# Problem shape (hardcoded per harness contract).
VOCAB = 100000
CTX = 8
EMB = 64
BATCH = 2048
K = CTX * EMB            # 512 contraction dim
NCORES = 8
VSHARD = 12544           # 98 * 128, vocab cols per core (padded)
VPAD = VSHARD * NCORES   # 100352
NVT = VSHARD // 128      # 98 vocab tiles per core
VCHUNK = 7               # vocab tiles per fc DMA chunk
NCHUNK = NVT // VCHUNK   # 14
CHUNK_COLS = VCHUNK * 128  # 896
NBT = BATCH // 128       # 16 batch tiles
NBC = 4                  # batch chunks (512 each) — outer loop
MPB = NBT // NBC         # batch tiles per chunk

F32 = mybir.dt.float32
BF16 = mybir.dt.bfloat16
FP8 = mybir.dt.float8e4
I32 = mybir.dt.int32
OUT_DT = BF16

E4NP = ml_dtypes.float8_e4m3   # TRN FP8_EXP4 semantics (max 240, inf above)
BFNP = ml_dtypes.bfloat16

SE = 2.0 ** 9            # embedding scale (pow2: exact descale)
SW = 2.0 ** 9            # fc weight scale
DESCALE = 1.0 / (SE * SW)

_CACHE = {}


def _install_trace_hook():
    """Provide the missing antenv.axon_hooks module so trace=True works."""
    import sys
    import types

    try:
        if "antenv.axon_hooks" not in sys.modules:
            mod = types.ModuleType("antenv.axon_hooks")
            mod._hook = None
            mod.set_axon_ntff_profile_hook = lambda h: setattr(mod, "_hook", h)
            mod.get_axon_ntff_profile_hook = lambda: mod._hook
            sys.modules["antenv.axon_hooks"] = mod
            import antenv

            antenv.axon_hooks = mod
        mod = sys.modules["antenv.axon_hooks"]
        if mod.get_axon_ntff_profile_hook() is None:
            if "/root/.axon_site/trn_agent_boot" not in sys.path:
                sys.path.insert(0, "/root/.axon_site/trn_agent_boot")
            import trn_boot

            mod.set_axon_ntff_profile_hook(
                trn_boot._ntff_profile_via_ctypes("/opt/axon/libaxon_pjrt.so")
            )
        return True
    except Exception as e:  # pragma: no cover
        print(f"trace hook install failed: {type(e).__name__}: {e}")
        return False


def _build_nc():
    nc = bacc.Bacc(
        "TRN2", target_bir_lowering=False, debug=False, num_devices=NCORES
    )
    # idx_all[p, j] = contexts[(j//8)*128 + p, j%8]  (j = m*8+c), same all cores
    idx_all = nc.declare_dram_parameter("idx_all", [128, 128], I32, isOutput=False)
    emb_wt = nc.declare_dram_parameter("emb_wt", [VOCAB, EMB], BF16, isOutput=False)
    # fc_w[ci, i, kg, par, w]: e4m3( SW * fc_W.T[kg*256+par*128+i, shard v] )
    fc_w = nc.declare_dram_parameter(
        "fc_w", [NCHUNK, 128, 2, 2, CHUNK_COLS], FP8, isOutput=False
    )
    fc_be = nc.declare_dram_parameter("fc_be", [128, NVT], F32, isOutput=False)
    out = nc.declare_dram_parameter(
        "out", [NBC, VSHARD, 512], OUT_DT, isOutput=True
    )

    with tile.TileContext(nc) as tc:
        with (
            tc.tile_pool(name="const", bufs=1) as const,
            tc.tile_pool(name="tpsum", bufs=2, space="PSUM") as tps,
            tc.tile_pool(name="mpsum", bufs=6, space="PSUM") as mps,
            tc.tile_pool(name="outp", bufs=8) as outp,
        ):
            idx_sb = const.tile([128, 128], I32, tag="idx", name="idx_sb")
            nc.sync.dma_start(out=idx_sb[:], in_=idx_all[:])
            fcbe_sb = const.tile([128, NVT], F32, tag="fcbe", name="fcbe_sb")
            nc.sync.dma_start(out=fcbe_sb[:], in_=fc_be[:])
            ident = const.tile([128, 128], BF16, tag="ident", name="ident")
            make_identity(nc, ident[:])
            # warm the ACT Identity table before the main loop needs it
            actwarm = const.tile([128, 1], F32, tag="actwarm", name="actwarm")
            nc.scalar.activation(
                out=actwarm[:],
                in_=fcbe_sb[:, 0:1],
                func=mybir.ActivationFunctionType.Identity,
                bias=fcbe_sb[:, 0:1],
            )

            # resident fc weights: [128 i, ci, kg, par, w] fp8 (49 KB/part)
            fcsb = const.tile(
                [128, NCHUNK, 2, 2, CHUNK_COLS], FP8, tag="fcsb", name="fcsb"
            )
            for ci in range(NCHUNK):
                nc.sync.dma_start(out=fcsb[:, ci], in_=fc_w[ci])

            # full-batch gather: raw16[p, j*64+e] = emb_wt[idx[p, j], e].
            # HW honors only ONE offset column per indirect call (multi-col
            # offset APs gather consecutive rows — verified on silicon), so
            # issue 128 single-column calls; bc0 needs only the first 32.
            raw16 = const.tile([128, NBT * K], BF16, tag="raw16", name="raw16")
            for j in range(NBT * CTX):
                nc.gpsimd.indirect_dma_start(
                    out=raw16[:, j * EMB : (j + 1) * EMB],
                    out_offset=None,
                    in_=emb_wt[:],
                    in_offset=bass.IndirectOffsetOnAxis(
                        ap=idx_sb[:, j : j + 1], axis=0
                    ),
                )

            # pair-interleaved fp8 moving operand:
            # embT8[kg][i, n, par] = e4m3(embed_scaled[kg*256+par*128+i, n])
            embT8 = [
                const.tile([128, BATCH, 2], FP8, tag=f"embT8{g}", name=f"embT8{g}")
                for g in range(2)
            ]

            def emit_transposes(bc):
                """PE transpose + DVE copy-cast building embT8 cols for bc."""
                for ml in range(MPB):
                    m = bc * MPB + ml
                    for kb in range(4):
                        ps = tps.tile([128, 1024], BF16, tag="tps", name="tps")
                        nc.tensor.transpose(
                            ps[:, 0:128],
                            raw16[:, m * K + kb * 128 : m * K + (kb + 1) * 128],
                            ident[:],
                        )
                        kg, par = kb // 2, kb % 2
                        nc.vector.tensor_copy(
                            out=embT8[kg][:, m * 128 : (m + 1) * 128, par],
                            in_=ps[:, 0:128],
                        )

            emit_transposes(0)
            for bc in range(NBC):
                # next chunk's transposes interleave into this pass's tail so
                # the PE never idles >3.4us (HAM stays at full clock)
                tp_queue = (
                    list(range(NBC * MPB * 4))[
                        (bc + 1) * MPB * 4 : (bc + 2) * MPB * 4
                    ]
                    if bc + 1 < NBC
                    else []
                )
                for v in range(NVT):
                    ci, vt = v // VCHUNK, v % VCHUNK
                    psm = mps.tile([128, 512], F32, tag="mps", name="mps")
                    for kg in range(2):
                        nc.tensor.matmul(
                            out=psm[:],
                            lhsT=fcsb[:, ci, kg, :, vt * 128 : (vt + 1) * 128],
                            rhs=embT8[kg][
                                :, bc * 512 : (bc + 1) * 512, :
                            ].rearrange("p n t -> p t n"),
                            start=(kg == 0),
                            stop=(kg == 1),
                            perf_mode=mybir.MatmulPerfMode.DoubleRow,
                        )
                    if v % 4 == 0:
                        osb4 = outp.tile(
                            [128, 4, 512], OUT_DT, tag="osb4", name="osb4"
                        )
                    if v % 2 == 0:
                        nc.scalar.activation(
                            out=osb4[:, v % 4, :],
                            in_=psm[:],
                            func=mybir.ActivationFunctionType.Identity,
                            bias=fcbe_sb[:, v : v + 1],
                            scale=DESCALE,
                        )
                    else:
                        nc.vector.tensor_scalar(
                            out=osb4[:, v % 4, :],
                            in0=psm[:],
                            scalar1=DESCALE,
                            scalar2=fcbe_sb[:, v : v + 1],
                            op0=mybir.AluOpType.mult,
                            op1=mybir.AluOpType.add,
                        )
                    if v % 4 == 3 or v == NVT - 1:
                        cnt = v % 4 + 1
                        nc.sync.dma_start(
                            out=out[
                                bc, (v - cnt + 1) * 128 : (v + 1) * 128, :
                            ].rearrange("(i p) c -> p i c", p=128),
                            in_=osb4[:, 0:cnt, :],
                        )
                    # trickle next chunk's 16 transposes through the last
                    # third of this pass (gathers are long done by then)
                    if tp_queue and v >= 64 and (v - 64) % 2 == 0:
                        ti = (v - 64) // 2
                        if ti < MPB * 4:
                            m = (bc + 1) * MPB + ti // 4
                            kb = ti % 4
                            ps = tps.tile(
                                [128, 1024], BF16, tag="tps", name="tps"
                            )
                            nc.tensor.transpose(
                                ps[:, 0:128],
                                raw16[
                                    :,
                                    m * K + kb * 128 : m * K + (kb + 1) * 128,
                                ],
                                ident[:],
                            )
                            kg, par = kb // 2, kb % 2
                            nc.vector.tensor_copy(
                                out=embT8[kg][:, m * 128 : (m + 1) * 128, par],
                                in_=ps[:, 0:128],
                            )
    nc.compile()
    return nc


def _prep_inputs(contexts, emb_W, emb_b, fc_W, fc_b):
    contexts = np.asarray(contexts)
    emb_W = np.asarray(emb_W, dtype=np.float32)
    emb_b = np.asarray(emb_b, dtype=np.float32)
    fc_W = np.asarray(fc_W, dtype=np.float32)
    fc_b = np.asarray(fc_b, dtype=np.float32)

    # idx_all[p, j] = contexts[(j//8)*128 + p, j%8]
    idx2d = (
        contexts.astype(np.int64).reshape(NBT, 128, CTX).transpose(0, 2, 1)
        .reshape(NBT * CTX, 128)
    )
    idx_all = np.ascontiguousarray(idx2d.T.astype(np.int32))

    # scaled bf16 embedding table (device casts bf16 -> e4m3 during drain copy)
    emb_wt = np.ascontiguousarray(
        np.clip(emb_W.T * SE, -240.0, 240.0).astype(BFNP)
    )

    # effective bias: fc_be = fc_W @ tile(emb_b, CTX) + fc_b  (exact, padded)
    emb_b_t = np.tile(emb_b, CTX)
    fc_be_full = (
        fc_W.astype(np.float64) @ emb_b_t.astype(np.float64)
        + fc_b.astype(np.float64)
    ).astype(np.float32)
    fc_be_pad = np.zeros(VPAD, dtype=np.float32)
    fc_be_pad[:VOCAB] = fc_be_full

    # fc_W.T scaled/quantized to e4m3, padded to VPAD cols
    fcT = np.zeros((K, VPAD), dtype=np.float32)
    fcT[:, :VOCAB] = fc_W.T
    fcq = np.clip(fcT * SW, -240.0, 240.0).astype(E4NP)

    in_maps = []
    for s in range(NCORES):
        shard = fcq[:, s * VSHARD : (s + 1) * VSHARD]
        # [k=kg*256+par*128+i, v=ci*896+w] -> [ci, i, kg, par, w]
        fc_host = np.ascontiguousarray(
            shard.reshape(2, 2, 128, NCHUNK, CHUNK_COLS).transpose(3, 2, 0, 1, 4)
        )
        be = np.ascontiguousarray(
            fc_be_pad[s * VSHARD : (s + 1) * VSHARD].reshape(NVT, 128).T
        )
        in_maps.append(
            {"idx_all": idx_all, "emb_wt": emb_wt, "fc_w": fc_host, "fc_be": be}
        )
    return in_maps


def kernel(contexts, emb_W, emb_b, fc_W, fc_b):
    if "nc" not in _CACHE:
        _CACHE["nc"] = _build_nc()
    nc = _CACHE["nc"]
    in_maps = _prep_inputs(contexts, emb_W, emb_b, fc_W, fc_b)
    trace = bool(int(os.environ.get("KERNEL_TRACE", "0")))
    if trace:
        trace = _install_trace_hook()
    res = run_bass_kernel_spmd(
        nc, in_maps, core_ids=list(range(NCORES)), trace=trace
    )
    _CACHE["last_exec_time_ns"] = res.exec_time_ns
    # out[s][bc, v*128+r, j] = logits[bc*512+j, s*VSHARD + v*128+r]
    full = np.empty((BATCH, VPAD), dtype=np.float32)
    for s, r in enumerate(res.results):
        o = np.asarray(r["out"]).astype(np.float32)  # [NBC, VSHARD, 512]
        for bc in range(NBC):
            full[bc * 512 : (bc + 1) * 512, s * VSHARD : (s + 1) * VSHARD] = o[
                bc
            ].T
    return np.ascontiguousarray(full[:, :VOCAB])
